# revision 35
# baseline (speedup 1.0000x reference)
"""GraphSAGE 2-layer GNN on TRN2, 8-core SPMD Bass/Tile kernel (v3).

Strategy (v3 — minimal host I/O):
- Nodes sharded across 8 cores (6250 each). Edge slots sorted by
  (dst tile, src region, src parity, src id), padded to 128-slot chunks with
  a per-(tile,group) budget equal across cores (SPMD uniform).
- x is NOT host-gathered: each core ships only its own x shard (bf16,
  [2*nhr, 128]: per-tile 64-row even block then odd block). Two AllGathers
  per region replicate x across cores on-device; layer-1 messages are then
  fetched by dma_gather — with the SAME int16 index table layer 2 uses,
  because the x tables use the identical per-tile 64-row pair layout as the
  z2 pair table (row = tile*64 + loc//2, table chosen by src parity).
- Segment-sum via one-hot matmul on PE. One-hots are HOST-PRECOMPUTED fp8
  constants resident in SBUF, device-cached across calls (edge-structure
  constants are uploaded once per unique edge_index, not per call).
- Layer 2: z = h @ Wl2 (40 cols padded to 64, bf16) packed in node PAIRS:
  z2 row r = [z[2r] | z[2r+1]] (256B rows), AllGathered per region, rows
  fetched by dma_gather (win slots per call), chunk parity selects column
  half.
- Per-tile PSUM scale/copies run on the Activation engine (per-partition
  invc scale AP); PE does transposes + dense matmuls in bf16.
"""
from dataclasses import dataclass, field
import numpy as np
import ml_dtypes

import concourse.bacc as bacc
import concourse.bass as bass
import concourse.mybir as mybir
import concourse.tile as tile
from concourse import library_config

P = 128
FP8 = ml_dtypes.float8_e4m3
BF16 = ml_dtypes.bfloat16


@dataclass
class Plan:
    n_nodes: int
    n_feat: int
    n_hid: int
    n_class: int
    n_cores: int
    npc: int                 # nodes per core
    nt: int                  # dst tiles per core
    nhr: int                 # pair-rows per core (nt*64)
    win: int                 # dma_gather window (slots)
    regions: list            # region boundaries in tiles, e.g. [49]
    budget: np.ndarray       # [nt, nreg*2] chunks per (tile, group)
    nch: int = 0             # total chunks per core per layer
    S: int = 0               # total slots (nch*128)
    chunk_par: np.ndarray = None    # [nch] parity
    chunk_reg: np.ndarray = None    # [nch] region
    chunk_spi: np.ndarray = None    # [nch] index within (reg,par) stream
    SRP: list = field(default_factory=list)        # [nreg][2] slots per stream
    OFF: list = field(default_factory=list)        # [nreg][2] slot offset in idx
    oh_tab: list = field(default_factory=list)     # [128, nch, 128] fp8
    idx: list = field(default_factory=list)        # per core [128, S//16] int16
    src_slot: list = field(default_factory=list)   # [S] int64 (-1 pad)
    invc_perm: list = field(default_factory=list)  # [128, nt] f32


def _wrap_idx(arr_i16: np.ndarray) -> np.ndarray:
    # position j -> partition j%16, col j//16; replicated 8x down partitions
    w = arr_i16.reshape(-1, 16).T            # [16, n/16]
    return np.ascontiguousarray(np.tile(w, (8, 1)))  # [128, n/16]


def make_plan(edge_index: np.ndarray, n_nodes: int, n_feat: int, n_hid: int,
              n_class: int, n_cores: int, win: int = 1024,
              regions: list | None = None) -> Plan:
    src = np.asarray(edge_index[0], dtype=np.int64)
    dst = np.asarray(edge_index[1], dtype=np.int64)
    npc = n_nodes // n_cores
    assert npc * n_cores == n_nodes and npc % 2 == 0
    nt = (npc + P - 1) // P
    nhr = nt * 64

    deg = np.bincount(dst, minlength=n_nodes).astype(np.float64)
    invc = (1.0 / np.maximum(deg, 1.0)).astype(np.float32)

    core_of = dst // npc
    rem = dst - core_of * npc
    tloc = rem // P
    loc = rem - tloc * P
    dp = (loc >> 1) + 64 * (loc & 1)           # permuted one-hot column
    par = (src & 1).astype(np.int64)            # parity of src

    if regions is None:
        regions = [nt]
    assert regions[-1] == nt
    nreg = len(regions)
    rb_rows = np.array([0] + regions) * 64      # pair-row boundaries per core

    # src pair-row within its core: trow = tile*64 + loc//2 == (src % npc)>>1
    sc = src // npc
    trow = (src - sc * npc) >> 1
    src_reg = np.searchsorted(rb_rows[1:], trow, side="right")
    grp = src_reg * 2 + par                     # group id per edge
    ng = nreg * 2

    counts = np.zeros((n_cores, nt, ng), dtype=np.int64)
    np.add.at(counts, (core_of, tloc, grp), 1)
    budget = np.ceil(counts.max(axis=0) / P).astype(np.int64)  # [nt, ng]
    nch = int(budget.sum())
    S = nch * P

    plan = Plan(n_nodes=n_nodes, n_feat=n_feat, n_hid=n_hid, n_class=n_class,
                n_cores=n_cores, npc=npc, nt=nt, nhr=nhr, win=win,
                regions=list(regions), budget=budget, nch=nch, S=S)

    # chunk offsets per (t, g) in chunk units (global chunk order)
    flat = budget.reshape(-1)
    chunk_off = np.concatenate([[0], np.cumsum(flat)])[:-1].reshape(nt, ng)
    chunk_par = np.zeros(nch, np.int8)
    chunk_reg = np.zeros(nch, np.int8)
    chunk_spi = np.zeros(nch, np.int64)
    spc = [[0] * 2 for _ in range(nreg)]
    for t in range(nt):
        for g in range(ng):
            r, p = g // 2, g % 2
            o = chunk_off[t, g]
            nb = int(budget[t, g])
            chunk_par[o:o + nb] = p
            chunk_reg[o:o + nb] = r
            chunk_spi[o:o + nb] = np.arange(spc[r][p], spc[r][p] + nb)
            spc[r][p] += nb
    plan.chunk_par, plan.chunk_reg, plan.chunk_spi = chunk_par, chunk_reg, chunk_spi
    plan.SRP = [[spc[r][0] * P, spc[r][1] * P] for r in range(nreg)]
    off = 0
    plan.OFF = []
    for r in range(nreg):
        o0 = off
        off += plan.SRP[r][0]
        o1 = off
        off += plan.SRP[r][1]
        plan.OFF.append([o0, o1])
    assert off == S

    # sort edges by (core, tile, group, src) — src order gives the dma_gather
    # descriptor stream HBM locality
    key = core_of * (nt * ng) + tloc * ng + grp
    order = np.lexsort((src, key))
    srcg = src[order]; keyg = key[order]
    dpg = dp[order]

    rr_of = np.diff(rb_rows)                    # pair-rows per region
    for c in range(n_cores):
        lo = np.searchsorted(keyg, c * nt * ng, side="left")
        hi = np.searchsorted(keyg, (c + 1) * nt * ng, side="left")
        sel = slice(lo, hi)
        st = srcg[sel]; kt = keyg[sel] - c * nt * ng; dt_ = dpg[sel]

        src_slot = np.full(S, -1, np.int64)
        dp_slot = np.full(S, -1, np.int64)
        bounds = np.concatenate([[0], np.where(np.diff(kt) != 0)[0] + 1, [len(st)]])
        for b0, b1 in zip(bounds[:-1], bounds[1:]):
            k = int(kt[b0]); t = k // ng; g = k % ng
            o = int(chunk_off[t, g]) * P
            n = b1 - b0
            src_slot[o:o + n] = st[b0:b1]
            dp_slot[o:o + n] = dt_[b0:b1]

        # one-hot table fp8: [128 slot-partitions, nch, 128]
        oh = np.zeros((P, nch, P), FP8)
        s_all = np.arange(S)
        valid = dp_slot >= 0
        oh[s_all[valid] % P, s_all[valid] // P, dp_slot[valid]] = 1.0
        plan.oh_tab.append(np.ascontiguousarray(oh))

        # unified idx table: row within region table = sc*rr + (trow - rb)
        sv = np.where(src_slot >= 0, src_slot, 0)
        svc = sv // npc
        svt = (sv % npc) >> 1
        svr = np.searchsorted(rb_rows[1:], svt, side="right")
        idx_all = svc * rr_of[svr] + (svt - rb_rows[svr])
        idx_cols = []
        for r in range(nreg):
            for p in range(2):
                chunks_rp = np.where((chunk_reg == r) & (chunk_par == p))[0]
                slot_sel = (chunks_rp[:, None] * P + np.arange(P)[None, :]).reshape(-1)
                vals = idx_all[slot_sel]
                pad = src_slot[slot_sel] < 0
                vals = np.where(pad, 0, vals)
                assert len(vals) == 0 or vals.max() < 32768
                if len(vals):
                    idx_cols.append(_wrap_idx(vals.astype(np.int16)))
        plan.idx.append(np.ascontiguousarray(np.hstack(idx_cols)))
        plan.src_slot.append(src_slot)

        # permuted invc: partition p<64 -> loc 2p ; p>=64 -> loc 2(p-64)+1
        ic = np.zeros((P, nt), np.float32)
        base = c * npc
        for t in range(nt):
            rows = min(P, npc - t * P)
            locs = np.concatenate([np.arange(0, rows, 2), np.arange(1, rows, 2)])
            pos = np.concatenate([np.arange(0, (rows + 1) // 2),
                                  64 + np.arange(0, rows // 2)])
            ic[pos, t] = invc[base + t * P + locs]
        plan.invc_perm.append(ic)
    return plan


CONST_NAMES = ("oh_tab", "idx", "invc", "ident", "identb")
X_NAMES = ("xsh", "wl1", "wr1", "wl2p", "wr2", "b1", "b2")


def stage_const(plan: Plan):
    """Edge-structure constants — uploaded once per unique edge_index."""
    ident = np.eye(P, dtype=np.float32)
    identb = np.eye(P, dtype=BF16)
    return [{"oh_tab": plan.oh_tab[c], "idx": plan.idx[c],
             "invc": plan.invc_perm[c], "ident": ident, "identb": identb}
            for c in range(plan.n_cores)]


def stage_x(plan: Plan, x, Wl1, Wr1, b1, Wl2, Wr2, b2):
    """Per-call inputs: x shards (pair-layout, bf16) + weights."""
    n, f = x.shape
    hid = plan.n_hid
    ncl = plan.n_class
    npc, nt, nhr = plan.npc, plan.nt, plan.nhr
    x_bf = np.asarray(x, dtype=np.float32).astype(BF16)
    wl1 = np.asarray(Wl1, np.float32).astype(BF16)
    wr1 = np.asarray(Wr1, np.float32).astype(BF16)
    wl2p = np.zeros((hid, 64), BF16)
    wl2p[:, :ncl] = np.asarray(Wl2, np.float32).astype(BF16)
    wr2 = np.asarray(Wr2, np.float32).astype(BF16)
    b1c = np.asarray(b1, np.float32).reshape(hid, 1)
    b2bc = np.broadcast_to(np.asarray(b2, np.float32), (P, ncl)).copy()

    in_maps = []
    for c in range(plan.n_cores):
        xp = np.zeros((nt * P, f), BF16)
        xp[:npc] = x_bf[c * npc:(c + 1) * npc]
        # pair layout: row t*64+j = [x[t*128+2j] | x[t*128+2j+1]] (256 cols),
        # mirroring the z2 pair table so one idx table serves both layers
        xsh = xp.reshape(nhr, 2 * f)
        in_maps.append({
            "xsh": np.ascontiguousarray(xsh),
            "wl1": wl1, "wr1": wr1, "wl2p": wl2p, "wr2": wr2,
            "b1": b1c, "b2": b2bc,
        })
    return in_maps


def build_program(plan: Plan, repeats: int = 1, parts: str = "full",
                  single_packet: bool = True, m1_bufs: int = 4,
                  m2_bufs: int = 6, z_splits: list | None = None):
    # parts: "xag" = x allgathers only; "l1" = + layer1; "l1ag" = + z2
    # allgathers; "full" = everything
    do_l1 = parts in ("l1", "l1ag", "full")
    do_ag = parts in ("l1ag", "full")
    do_l2 = parts == "full"
    f = plan.n_feat
    hid = plan.n_hid
    ncl = plan.n_class
    nt = plan.nt
    npc = plan.npc
    nhr = plan.nhr
    nch = plan.nch
    S = plan.S
    win = plan.win
    f32 = mybir.dt.float32
    bf16 = mybir.dt.bfloat16
    fp8 = mybir.dt.float8e4
    ncores = plan.n_cores
    budget = plan.budget

    nc = bacc.Bacc("TRN2", target_bir_lowering=False, debug=False,
                   enable_asserts=False, num_devices=ncores,
                   num_swdge_queues=4)

    regions = plan.regions
    nreg = len(regions)
    rb_tiles = [0] + regions
    rb_rows = [b * 64 for b in rb_tiles]
    rr_of = [rb_rows[r + 1] - rb_rows[r] for r in range(nreg)]
    SRP = plan.SRP
    OFF = plan.OFF
    chunk_par = plan.chunk_par
    chunk_reg = plan.chunk_reg
    chunk_spi = plan.chunk_spi

    if z_splits is None:
        z_splits = [nt]
    assert z_splits[-1] == nt

    oh_d = nc.dram_tensor("oh_tab", [P, nch, P], fp8, kind="ExternalInput")
    idx_d = nc.dram_tensor("idx", [P, S // 16], mybir.dt.int16,
                           kind="ExternalInput")
    invc_d = nc.dram_tensor("invc", [P, nt], f32, kind="ExternalInput")
    xsh_d = nc.dram_tensor("xsh", [nhr, 2 * f], bf16, kind="ExternalInput")
    wl1_d = nc.dram_tensor("wl1", [f, hid], bf16, kind="ExternalInput")
    wr1_d = nc.dram_tensor("wr1", [f, hid], bf16, kind="ExternalInput")
    wl2p_d = nc.dram_tensor("wl2p", [hid, 64], bf16, kind="ExternalInput")
    wr2_d = nc.dram_tensor("wr2", [hid, ncl], bf16, kind="ExternalInput")
    b1_d = nc.dram_tensor("b1", [hid, 1], f32, kind="ExternalInput")
    b2_d = nc.dram_tensor("b2", [P, ncl], f32, kind="ExternalInput")
    ident_d = nc.dram_tensor("ident", [P, P], f32, kind="ExternalInput")
    identb_d = nc.dram_tensor("identb", [P, P], bf16, kind="ExternalInput")
    out_d = nc.dram_tensor("out", [npc, ncl], f32, kind="ExternalOutput")

    with tile.TileContext(nc) as tc:
        nc.gpsimd.load_library(library_config.mlp)
        with tc.tile_pool(name="const", bufs=1) as cp, \
             tc.tile_pool(name="store", bufs=1) as sp, \
             tc.tile_pool(name="m1", bufs=m1_bufs) as mp1, \
             tc.tile_pool(name="m2", bufs=m2_bufs) as mp2, \
             tc.tile_pool(name="xr", bufs=2) as xrp, \
             tc.tile_pool(name="fin", bufs=2) as fp, \
             tc.tile_pool(name="seg", bufs=2, space="PSUM") as psum_seg, \
             tc.tile_pool(name="paux", bufs=2, space="PSUM") as psum_aux, \
             tc.tile_pool(name="px", bufs=2, space="PSUM") as psum_x, \
             tc.tile_pool(name="phT", bufs=2, space="PSUM") as psum_h, \
             tc.tile_pool(name="dram", bufs=1, space="DRAM") as dp:

            # ---- constant staging ----
            def load_const(dram, shape, dtype=f32, tag="", slices=1):
                t = cp.tile(shape, dtype, tag=tag)
                if slices == 1:
                    nc.sync.dma_start(t[:], dram[:])
                else:
                    step = shape[1] // slices
                    for i in range(slices):
                        sl = slice(i * step, (i + 1) * step if i < slices - 1 else shape[1])
                        nc.sync.dma_start(t[:, sl], dram[:, sl])
                return t

            oh_t = load_const(oh_d, [P, nch, P], fp8, tag="oh", slices=8)
            idx_t = load_const(idx_d, [P, S // 16], mybir.dt.int16, tag="idx")
            invc_t = load_const(invc_d, [P, nt], tag="invc")
            wl1_t = load_const(wl1_d, [f, hid], bf16, tag="wl1")
            wr1_t = load_const(wr1_d, [f, hid], bf16, tag="wr1")
            wl2p_t = load_const(wl2p_d, [hid, 64], bf16, tag="wl2p")
            wr2_t = load_const(wr2_d, [hid, ncl], bf16, tag="wr2")
            b1_t = load_const(b1_d, [hid, 1], tag="b1")
            b2_t = load_const(b2_d, [P, ncl], tag="b2")
            ident_t = load_const(ident_d, [P, P], tag="ident")
            identb_t = load_const(identb_d, [P, P], bf16, tag="identb")

            hT_store = sp.tile([P, nt * P], bf16, tag="hT")     # [hid, node']
            z_acc = sp.tile([P, nt, 64], bf16, tag="z_acc")     # [node', tile, zcol]
            out_acc = sp.tile([P, nt, ncl], f32, tag="out_acc")
            s2_acc = (sp.tile([P, nt, ncl], f32, tag="s2_acc", name="s2_acc")
                      if nreg > 1 else None)

            # global chunk ids per (t, g)
            flatb = budget.reshape(-1)
            chunk_off = np.concatenate([[0], np.cumsum(flatb)])[:-1].reshape(nt, 2 * nreg)

            # collectives cannot read IO tensors: mirror xsh into internal DRAM
            xsh_i = dp.tile([nhr, 2 * f], bf16, name="xsh_i")
            nc.sync.dma_start(xsh_i[:, :], xsh_d[:, :])

            for _rep in range(repeats):
                x_full = [dp.tile([ncores, rr_of[r], 2 * f], bf16,
                                  addr_space="Shared", name=f"xf{r}")
                          for r in range(nreg)]
                z2_own = dp.tile([nhr, P], bf16)
                z2_full = [dp.tile([ncores, rr_of[r], P], bf16,
                                   addr_space="Shared", name=f"z2full{r}")
                           for r in range(nreg)]

                # replicate x across cores: ONE AllGather per region of the
                # 512B pair-rows; gathers fetch the full row (contiguous) and
                # the matmul picks the parity half
                for r in range(nreg):
                    r0, r1 = rb_rows[r], rb_rows[r + 1]
                    nc.gpsimd.collective_compute(
                        "AllGather", mybir.AluOpType.bypass,
                        replica_groups=[list(range(ncores))],
                        ins=[xsh_i[r0:r1, :]], outs=[x_full[r][:, :, :]])

                if not do_l1:
                    continue

                x_tabs = [x_full[r][:, :, :].flatten_outer_dims()
                          for r in range(nreg)]
                z_tabs = [z2_full[r][:, :, :].flatten_outer_dims()
                          for r in range(nreg)]

                qrot = [0]
                msg_bufs1 = {}
                msg_bufs2 = {}

                def ensure_win(layer, r, p, w):
                    cache = msg_bufs1 if layer == 1 else msg_bufs2
                    if (r, p, w) in cache:
                        return cache[(r, p, w)]
                    lo = w * win
                    cnt = min(win, SRP[r][p] - lo)
                    pool = mp1 if layer == 1 else mp2
                    tbl = x_tabs[r] if layer == 1 else z_tabs[r]
                    es = 2 * f if layer == 1 else P
                    mt = pool.tile([P, win // P, es], bf16, tag=f"m{layer}")
                    c0 = (OFF[r][p] + lo) // 16
                    nc.gpsimd.dma_gather(
                        mt[:, :cnt // P, :], tbl,
                        idx_t[:, c0:c0 + cnt // 16], cnt, cnt, es,
                        queue_num=qrot[0] % 4, single_packet=single_packet)
                    qrot[0] += 1
                    cache[(r, p, w)] = mt
                    return mt

                # ================= LAYER 1 =================
                ci = 0
                t_done = 0
                for t in range(nt):
                    ncht = int(budget[t, :].sum())
                    pt = psum_seg.tile([P, f], f32, tag="seg")
                    for j in range(ncht):
                        r = int(chunk_reg[ci]); p = int(chunk_par[ci])
                        w, col = divmod(int(chunk_spi[ci]) * P, win)
                        mt = ensure_win(1, r, p, w)
                        pb = f * p
                        nc.tensor.matmul(out=pt[:], lhsT=oh_t[:, ci, :],
                                         rhs=mt[:, col // P, pb:pb + f],
                                         start=(j == 0), stop=(j == ncht - 1))
                        ci += 1
                    # mean scale on Act (per-partition invc), f32 out
                    aggm = fp.tile([P, f], f32, tag="aggm")
                    if ncht == 0:
                        nc.vector.memset(aggm[:], 0.0)
                    else:
                        nc.scalar.activation(out=aggm[:], in_=pt[:],
                                             func=mybir.ActivationFunctionType.Copy,
                                             scale=invc_t[:, t:t + 1])
                    paggT = psum_aux.tile([P, P], f32, tag="aux")
                    nc.tensor.transpose(out=paggT[:], in_=aggm[:], identity=ident_t[:])
                    aggT = fp.tile([P, P], bf16, tag="aggT_sb")
                    nc.scalar.activation(out=aggT[:], in_=paggT[:],
                                         func=mybir.ActivationFunctionType.Copy)
                    # own x tile -> transpose (pair layout: even half | odd half)
                    xr = xrp.tile([P, f], bf16, tag="xr")
                    nc.sync.dma_start(xr[0:64, :], xsh_d[t * 64:(t + 1) * 64, 0:f])
                    nc.sync.dma_start(xr[64:128, :],
                                      xsh_d[t * 64:(t + 1) * 64, f:2 * f])
                    pxT = psum_x.tile([P, P], bf16, tag="pxT")
                    nc.tensor.transpose(out=pxT[:], in_=xr[:], identity=identb_t[:])
                    xTt = fp.tile([P, P], bf16, tag="xT_sb")
                    nc.scalar.activation(out=xTt[:], in_=pxT[:],
                                         func=mybir.ActivationFunctionType.Copy)
                    phT = psum_h.tile([P, P], f32, tag="hT")
                    nc.tensor.matmul(out=phT[:], lhsT=wl1_t[:], rhs=aggT[:],
                                     start=True, stop=False)
                    nc.tensor.matmul(out=phT[:], lhsT=wr1_t[:], rhs=xTt[:],
                                     start=False, stop=True)
                    hT_sl = hT_store[:, t * P:(t + 1) * P]
                    nc.scalar.activation(out=hT_sl, in_=phT[:],
                                         func=mybir.ActivationFunctionType.Relu,
                                         bias=b1_t[:], scale=1.0)
                    pz = psum_aux.tile([P, P], f32, tag="aux")
                    nc.tensor.matmul(out=pz[:, 0:64], lhsT=hT_sl, rhs=wl2p_t[:],
                                     start=True, stop=True)
                    nc.scalar.activation(out=z_acc[:, t, :], in_=pz[:, 0:64],
                                         func=mybir.ActivationFunctionType.Copy)

                    t_done += 1
                    if t_done in regions:
                        k = regions.index(t_done)
                        r0, r1 = rb_rows[k], rb_rows[k + 1]
                        t0 = rb_tiles[k]
                        # z_acc[[par half], t0:t, :] -> z2_own rows, col half
                        for par, pbase in ((0, 0), (1, 64)):
                            src_ap = z_acc[pbase:pbase + 64, t0:t_done, :]
                            dst_ap = z2_own[r0:r1, pbase:pbase + 64]
                            d3 = dst_ap.rearrange("(t j) c -> j t c", j=64)
                            nc.sync.dma_start(d3, src_ap)
                        if do_ag:
                            nc.gpsimd.collective_compute(
                                "AllGather", mybir.AluOpType.bypass,
                                replica_groups=[list(range(ncores))],
                                ins=[z2_own[r0:r1, :]],
                                outs=[z2_full[k][:, :, :]])

                # ================= LAYER 2 =================
                if not do_l2:
                    continue

                def l2_chunks(t, ri):
                    ids = []
                    for g in (2 * ri, 2 * ri + 1):
                        o = int(chunk_off[t, g])
                        ids += list(range(o, o + int(budget[t, g])))
                    return ids

                # passes 0..nreg-2 accumulate region partial sums in SBUF so the
                # next region's AllGather latency hides under this pass's work
                for ri in range(nreg - 1):
                    for t in range(nt):
                        ids = l2_chunks(t, ri)
                        if not ids:
                            if ri == 0:
                                nc.vector.memset(s2_acc[:, t, :], 0.0)
                            continue
                        pt = psum_seg.tile([P, f], f32, tag="seg")
                        for j, ci2 in enumerate(ids):
                            p = int(chunk_par[ci2])
                            w, col = divmod(int(chunk_spi[ci2]) * P, win)
                            mt = ensure_win(2, ri, p, w)
                            pbase = 64 * p
                            nc.tensor.matmul(out=pt[:, 0:ncl], lhsT=oh_t[:, ci2, :],
                                             rhs=mt[:, col // P, pbase:pbase + ncl],
                                             start=(j == 0), stop=(j == len(ids) - 1))
                        if ri == 0:
                            nc.scalar.activation(out=s2_acc[:, t, :], in_=pt[:, 0:ncl],
                                                 func=mybir.ActivationFunctionType.Copy,
                                                 scale=invc_t[:, t:t + 1])
                        else:
                            s2p = fp.tile([P, ncl], f32, tag="s2")
                            nc.scalar.activation(out=s2p[:], in_=pt[:, 0:ncl],
                                                 func=mybir.ActivationFunctionType.Copy,
                                                 scale=invc_t[:, t:t + 1])
                            nc.vector.tensor_add(out=s2_acc[:, t, :],
                                                 in0=s2_acc[:, t, :], in1=s2p[:])

                rif = nreg - 1
                for t in range(nt):
                    ids = l2_chunks(t, rif)
                    pt = psum_seg.tile([P, f], f32, tag="seg")
                    for j, ci2 in enumerate(ids):
                        p = int(chunk_par[ci2])
                        w, col = divmod(int(chunk_spi[ci2]) * P, win)
                        mt = ensure_win(2, rif, p, w)
                        pbase = 64 * p
                        nc.tensor.matmul(out=pt[:, 0:ncl], lhsT=oh_t[:, ci2, :],
                                         rhs=mt[:, col // P, pbase:pbase + ncl],
                                         start=(j == 0), stop=(j == len(ids) - 1))
                    s2 = fp.tile([P, ncl], f32, tag="s2")
                    if not ids:
                        nc.vector.memset(s2[:], 0.0)
                    else:
                        nc.scalar.activation(out=s2[:], in_=pt[:, 0:ncl],
                                             func=mybir.ActivationFunctionType.Copy,
                                             scale=invc_t[:, t:t + 1])
                    po = psum_aux.tile([P, P], f32, tag="aux")
                    nc.tensor.matmul(out=po[:, 0:ncl], lhsT=hT_store[:, t * P:(t + 1) * P],
                                     rhs=wr2_t[:], start=True, stop=True)
                    ofin = out_acc[:, t, :]
                    nc.vector.tensor_add(out=ofin, in0=po[:, 0:ncl], in1=s2[:])
                    if nreg > 1:
                        nc.vector.tensor_add(out=ofin, in0=ofin, in1=s2_acc[:, t, :])
                    nc.vector.tensor_add(out=ofin, in0=ofin, in1=b2_t[:])

                # output write: per-tile strided (even/odd local) rows
                for t in range(nt):
                    rows = min(P, npc - t * P)
                    n_even = (rows + 1) // 2
                    n_odd = rows // 2
                    ev = out_acc[0:n_even, t, :]
                    od = out_acc[64:64 + n_odd, t, :]
                    dst_e = out_d[t * P:t * P + 2 * n_even - 1:2, :]
                    nc.sync.dma_start(dst_e, ev)
                    if n_odd:
                        dst_o = out_d[t * P + 1:t * P + 2 * n_odd:2, :]
                        nc.sync.dma_start(dst_o, od)

    nc.compile()
    return nc


import jax
from jax.sharding import Mesh, PartitionSpec, NamedSharding
from jax.experimental.shard_map import shard_map
from concourse.bass2jax import _bass_exec_p, partition_id_tensor, install_neuronx_cc_hook


class SpmdRunner:
    def __init__(self, nc, n_cores: int):
        install_neuronx_cc_hook()
        self.nc = nc
        self.n_cores = n_cores
        partition_name = nc.partition_id_tensor.name if nc.partition_id_tensor else None
        in_names, out_names, out_avals = [], [], []
        zero_outs = []
        for alloc in nc.m.functions[0].allocations:
            if not isinstance(alloc, mybir.MemoryLocationSet):
                continue
            name = alloc.memorylocations[0].name
            if alloc.kind == "ExternalInput":
                if name != partition_name:
                    in_names.append(name)
            elif alloc.kind == "ExternalOutput":
                shape = tuple(alloc.tensor_shape)
                dtype = mybir.dt.np(alloc.dtype)
                out_names.append(name)
                out_avals.append(jax.core.ShapedArray(shape, dtype))
                zero_outs.append(np.zeros(shape, dtype))
        self.in_names = list(in_names)
        self.out_names = out_names
        self.out_avals = out_avals
        self.zero_outs = zero_outs
        n_params = len(in_names)
        all_in_names = list(in_names) + list(out_names)
        if partition_name is not None:
            all_in_names.append(partition_name)

        def _body(*args):
            operands = list(args)
            if partition_name is not None:
                operands.append(partition_id_tensor())
            outs = _bass_exec_p.bind(
                *operands,
                out_avals=tuple(out_avals),
                in_names=tuple(all_in_names),
                out_names=tuple(out_names),
                lowering_input_output_aliases=(),
                sim_require_finite=False,
                sim_require_nnan=False,
                nc=nc,
            )
            return tuple(outs)

        devices = jax.devices()[:n_cores]
        assert len(devices) == n_cores
        self.mesh = Mesh(np.asarray(devices), ("core",))
        in_specs = (PartitionSpec("core"),) * (n_params + len(out_names))
        out_specs = (PartitionSpec("core"),) * len(out_names)
        self.fn = jax.jit(
            shard_map(_body, mesh=self.mesh, in_specs=in_specs,
                      out_specs=out_specs, check_rep=False),
            keep_unused=True,
        )
        self.sharding = NamedSharding(self.mesh, PartitionSpec("core"))
        self._dev = {}
        self._zero_dev = None

    def put(self, in_maps, names):
        """device_put the given input names (concatenated across cores)."""
        n = self.n_cores
        for name in names:
            a = np.concatenate([np.asarray(in_maps[c][name]) for c in range(n)],
                               axis=0)
            self._dev[name] = jax.device_put(a, self.sharding)
        if self._zero_dev is None:
            self._zero_dev = [
                jax.device_put(
                    np.zeros((n * z.shape[0], *z.shape[1:]), z.dtype),
                    self.sharding)
                for z in self.zero_outs
            ]
        return self

    def stage(self, in_maps):
        return self.put(in_maps, self.in_names)

    def _args(self):
        return [self._dev[n] for n in self.in_names] + self._zero_dev

    def run(self):
        return self.fn(*self._args())

    def run_blocking(self):
        out = self.fn(*self._args())
        jax.block_until_ready(out)
        return out

    def results(self, out_arrs):
        n = self.n_cores
        return [
            {name: np.asarray(out_arrs[i]).reshape(n, *self.out_avals[i].shape)[c]
             for i, name in enumerate(self.out_names)}
            for c in range(n)
        ]


# ---------------- self-contained entry point ----------------
_CACHE = {}

def kernel(**inputs):
    import numpy as _np
    x = _np.asarray(inputs["x"], dtype=_np.float32)
    edge_index = _np.asarray(inputs["edge_index"])
    Wl1 = _np.asarray(inputs["Wl1"], dtype=_np.float32)
    Wr1 = _np.asarray(inputs["Wr1"], dtype=_np.float32)
    b1 = _np.asarray(inputs["b1"], dtype=_np.float32)
    Wl2 = _np.asarray(inputs["Wl2"], dtype=_np.float32)
    Wr2 = _np.asarray(inputs["Wr2"], dtype=_np.float32)
    b2 = _np.asarray(inputs["b2"], dtype=_np.float32)
    N, F = x.shape
    H = Wl1.shape[1]
    C = Wl2.shape[1]
    import hashlib
    eh = hashlib.md5(edge_index.tobytes()).hexdigest()
    key = ("plan", N, F, H, C, edge_index.shape[1], eh)
    if key not in _CACHE:
        plan = make_plan(edge_index, N, F, H, C, 8)
        nc = build_program(plan)
        runner = SpmdRunner(nc, 8)
        runner.put(stage_const(plan), CONST_NAMES)
        _CACHE[key] = (plan, runner)
    plan, runner = _CACHE[key]
    runner.put(stage_x(plan, x, Wl1, Wr1, b1, Wl2, Wr2, b2), X_NAMES)
    out_arrs = runner.run_blocking()
    results = runner.results(out_arrs)
    out = _np.concatenate([results[c]["out"] for c in range(8)], axis=0)
    return out[:N].astype(_np.float32)


# revision 45
# speedup vs baseline: 1.3232x; 1.3232x over previous
"""GraphSAGE 2-layer GNN on TRN2, 8-core SPMD Bass/Tile kernel (v3).

Strategy (v3 — minimal host I/O):
- Nodes sharded across 8 cores (6250 each). Edge slots sorted by
  (dst tile, src region, src parity, src id), padded to 128-slot chunks with
  a per-(tile,group) budget equal across cores (SPMD uniform).
- x is NOT host-gathered: each core ships only its own x shard (bf16,
  [2*nhr, 128]: per-tile 64-row even block then odd block). Two AllGathers
  per region replicate x across cores on-device; layer-1 messages are then
  fetched by dma_gather — with the SAME int16 index table layer 2 uses,
  because the x tables use the identical per-tile 64-row pair layout as the
  z2 pair table (row = tile*64 + loc//2, table chosen by src parity).
- Segment-sum via one-hot matmul on PE. One-hots are HOST-PRECOMPUTED fp8
  constants resident in SBUF, device-cached across calls (edge-structure
  constants are uploaded once per unique edge_index, not per call).
- Layer 2: z = h @ Wl2 (40 cols padded to 64, bf16) packed in node PAIRS:
  z2 row r = [z[2r] | z[2r+1]] (256B rows), AllGathered per region, rows
  fetched by dma_gather (win slots per call), chunk parity selects column
  half.
- Per-tile PSUM scale/copies run on the Activation engine (per-partition
  invc scale AP); PE does transposes + dense matmuls in bf16.
"""
from dataclasses import dataclass, field
import numpy as np
import ml_dtypes

import concourse.bacc as bacc
import concourse.bass as bass
import concourse.mybir as mybir
import concourse.tile as tile
from concourse import library_config

P = 128
FP8 = ml_dtypes.float8_e4m3
BF16 = ml_dtypes.bfloat16


@dataclass
class Plan:
    n_nodes: int
    n_feat: int
    n_hid: int
    n_class: int
    n_cores: int
    npc: int                 # nodes per core
    nt: int                  # dst tiles per core
    nhr: int                 # pair-rows per core (nt*64)
    win: int                 # dma_gather window (slots)
    regions: list            # region boundaries in tiles, e.g. [49]
    budget: np.ndarray       # [nt, nreg*2] chunks per (tile, group)
    nch: int = 0             # total chunks per core per layer
    S: int = 0               # total slots (nch*128)
    chunk_par: np.ndarray = None    # [nch] parity
    chunk_reg: np.ndarray = None    # [nch] region
    chunk_spi: np.ndarray = None    # [nch] index within (reg,par) stream
    SRP: list = field(default_factory=list)        # [nreg][2] slots per stream
    OFF: list = field(default_factory=list)        # [nreg][2] slot offset in idx
    oh_tab: list = field(default_factory=list)     # [128, nch, 128] fp8
    idx: list = field(default_factory=list)        # per core [128, S//16] int16
    src_slot: list = field(default_factory=list)   # [S] int64 (-1 pad)
    invc_perm: list = field(default_factory=list)  # [128, nt] f32


def _wrap_idx(arr_i16: np.ndarray) -> np.ndarray:
    # position j -> partition j%16, col j//16; replicated 8x down partitions
    w = arr_i16.reshape(-1, 16).T            # [16, n/16]
    return np.ascontiguousarray(np.tile(w, (8, 1)))  # [128, n/16]


def make_plan(edge_index: np.ndarray, n_nodes: int, n_feat: int, n_hid: int,
              n_class: int, n_cores: int, win: int = 1024,
              regions: list | None = None) -> Plan:
    src = np.asarray(edge_index[0], dtype=np.int64)
    dst = np.asarray(edge_index[1], dtype=np.int64)
    npc = n_nodes // n_cores
    assert npc * n_cores == n_nodes and npc % 2 == 0
    nt = (npc + P - 1) // P
    nhr = nt * 64

    deg = np.bincount(dst, minlength=n_nodes).astype(np.float64)
    invc = (1.0 / np.maximum(deg, 1.0)).astype(np.float32)

    core_of = dst // npc
    rem = dst - core_of * npc
    tloc = rem // P
    loc = rem - tloc * P
    dp = (loc >> 1) + 64 * (loc & 1)           # permuted one-hot column
    par = (src & 1).astype(np.int64)            # parity of src

    if regions is None:
        regions = [nt]
    assert regions[-1] == nt
    nreg = len(regions)
    rb_rows = np.array([0] + regions) * 64      # pair-row boundaries per core

    # src pair-row within its core: trow = tile*64 + loc//2 == (src % npc)>>1
    sc = src // npc
    trow = (src - sc * npc) >> 1
    src_reg = np.searchsorted(rb_rows[1:], trow, side="right")
    grp = src_reg * 2 + par                     # group id per edge
    ng = nreg * 2

    counts = np.zeros((n_cores, nt, ng), dtype=np.int64)
    np.add.at(counts, (core_of, tloc, grp), 1)
    budget = np.ceil(counts.max(axis=0) / P).astype(np.int64)  # [nt, ng]
    nch = int(budget.sum())
    S = nch * P

    plan = Plan(n_nodes=n_nodes, n_feat=n_feat, n_hid=n_hid, n_class=n_class,
                n_cores=n_cores, npc=npc, nt=nt, nhr=nhr, win=win,
                regions=list(regions), budget=budget, nch=nch, S=S)

    # chunk offsets per (t, g) in chunk units (global chunk order)
    flat = budget.reshape(-1)
    chunk_off = np.concatenate([[0], np.cumsum(flat)])[:-1].reshape(nt, ng)
    chunk_par = np.zeros(nch, np.int8)
    chunk_reg = np.zeros(nch, np.int8)
    chunk_spi = np.zeros(nch, np.int64)
    spc = [[0] * 2 for _ in range(nreg)]
    for t in range(nt):
        for g in range(ng):
            r, p = g // 2, g % 2
            o = chunk_off[t, g]
            nb = int(budget[t, g])
            chunk_par[o:o + nb] = p
            chunk_reg[o:o + nb] = r
            chunk_spi[o:o + nb] = np.arange(spc[r][p], spc[r][p] + nb)
            spc[r][p] += nb
    plan.chunk_par, plan.chunk_reg, plan.chunk_spi = chunk_par, chunk_reg, chunk_spi
    plan.SRP = [[spc[r][0] * P, spc[r][1] * P] for r in range(nreg)]
    off = 0
    plan.OFF = []
    for r in range(nreg):
        o0 = off
        off += plan.SRP[r][0]
        o1 = off
        off += plan.SRP[r][1]
        plan.OFF.append([o0, o1])
    assert off == S

    # sort edges by (core, tile, group, src) — src order gives the dma_gather
    # descriptor stream HBM locality
    key = core_of * (nt * ng) + tloc * ng + grp
    order = np.lexsort((src, key))
    srcg = src[order]; keyg = key[order]
    dpg = dp[order]

    rr_of = np.diff(rb_rows)                    # pair-rows per region
    for c in range(n_cores):
        lo = np.searchsorted(keyg, c * nt * ng, side="left")
        hi = np.searchsorted(keyg, (c + 1) * nt * ng, side="left")
        sel = slice(lo, hi)
        st = srcg[sel]; kt = keyg[sel] - c * nt * ng; dt_ = dpg[sel]

        src_slot = np.full(S, -1, np.int64)
        dp_slot = np.full(S, -1, np.int64)
        bounds = np.concatenate([[0], np.where(np.diff(kt) != 0)[0] + 1, [len(st)]])
        for b0, b1 in zip(bounds[:-1], bounds[1:]):
            k = int(kt[b0]); t = k // ng; g = k % ng
            o = int(chunk_off[t, g]) * P
            n = b1 - b0
            src_slot[o:o + n] = st[b0:b1]
            dp_slot[o:o + n] = dt_[b0:b1]

        # one-hot table fp8: [128 slot-partitions, nch, 128]
        oh = np.zeros((P, nch, P), FP8)
        s_all = np.arange(S)
        valid = dp_slot >= 0
        oh[s_all[valid] % P, s_all[valid] // P, dp_slot[valid]] = 1.0
        plan.oh_tab.append(np.ascontiguousarray(oh))

        # unified idx table: row within region table = sc*rr + (trow - rb)
        sv = np.where(src_slot >= 0, src_slot, 0)
        svc = sv // npc
        svt = (sv % npc) >> 1
        svr = np.searchsorted(rb_rows[1:], svt, side="right")
        idx_all = svc * rr_of[svr] + (svt - rb_rows[svr])
        idx_cols = []
        for r in range(nreg):
            for p in range(2):
                chunks_rp = np.where((chunk_reg == r) & (chunk_par == p))[0]
                slot_sel = (chunks_rp[:, None] * P + np.arange(P)[None, :]).reshape(-1)
                vals = idx_all[slot_sel]
                pad = src_slot[slot_sel] < 0
                vals = np.where(pad, 0, vals)
                assert len(vals) == 0 or vals.max() < 32768
                if len(vals):
                    idx_cols.append(_wrap_idx(vals.astype(np.int16)))
        plan.idx.append(np.ascontiguousarray(np.hstack(idx_cols)))
        plan.src_slot.append(src_slot)

        # permuted invc: partition p<64 -> loc 2p ; p>=64 -> loc 2(p-64)+1
        ic = np.zeros((P, nt), np.float32)
        base = c * npc
        for t in range(nt):
            rows = min(P, npc - t * P)
            locs = np.concatenate([np.arange(0, rows, 2), np.arange(1, rows, 2)])
            pos = np.concatenate([np.arange(0, (rows + 1) // 2),
                                  64 + np.arange(0, rows // 2)])
            ic[pos, t] = invc[base + t * P + locs]
        plan.invc_perm.append(ic)
    return plan


CONST_NAMES = ("oh_tab", "idx", "invc", "ident", "identb")
X_NAMES = ("xsh", "wl1", "wr1", "wl2p", "wr2", "b1", "b2")


def stage_const(plan: Plan):
    """Edge-structure constants — uploaded once per unique edge_index."""
    ident = np.eye(P, dtype=np.float32)
    identb = np.eye(P, dtype=BF16)
    return [{"oh_tab": plan.oh_tab[c], "idx": plan.idx[c],
             "invc": plan.invc_perm[c], "ident": ident, "identb": identb}
            for c in range(plan.n_cores)]


def stage_x(plan: Plan, x, Wl1, Wr1, b1, Wl2, Wr2, b2):
    """Per-call inputs: x shards (pair-layout, bf16) + weights."""
    n, f = x.shape
    hid = plan.n_hid
    ncl = plan.n_class
    npc, nt, nhr = plan.npc, plan.nt, plan.nhr
    x_bf = np.asarray(x, dtype=np.float32).astype(BF16)
    wl1 = np.asarray(Wl1, np.float32).astype(BF16)
    wr1 = np.asarray(Wr1, np.float32).astype(BF16)
    wl2p = np.zeros((hid, 64), BF16)
    wl2p[:, :ncl] = np.asarray(Wl2, np.float32).astype(BF16)
    wr2 = np.asarray(Wr2, np.float32).astype(BF16)
    b1c = np.asarray(b1, np.float32).reshape(hid, 1)
    b2bc = np.broadcast_to(np.asarray(b2, np.float32), (P, ncl)).copy()

    in_maps = []
    for c in range(plan.n_cores):
        xp = np.zeros((nt * P, f), BF16)
        xp[:npc] = x_bf[c * npc:(c + 1) * npc]
        blk = xp.reshape(nt, P, f)
        xsh = np.concatenate([blk[:, 0::2, :].reshape(nhr, f),
                              blk[:, 1::2, :].reshape(nhr, f)], axis=0)
        in_maps.append({
            "xsh": np.ascontiguousarray(xsh),
            "wl1": wl1, "wr1": wr1, "wl2p": wl2p, "wr2": wr2,
            "b1": b1c, "b2": b2bc,
        })
    return in_maps


def build_program(plan: Plan, repeats: int = 1, parts: str = "full",
                  single_packet: bool = True, m1_bufs: int = 6,
                  m2_bufs: int = 6, z_splits: list | None = None):
    # parts: "xag" = x allgathers only; "l1" = + layer1; "l1ag" = + z2
    # allgathers; "full" = everything
    do_l1 = parts in ("l1", "l1ag", "full")
    do_ag = parts in ("l1ag", "full")
    do_l2 = parts == "full"
    f = plan.n_feat
    hid = plan.n_hid
    ncl = plan.n_class
    nt = plan.nt
    npc = plan.npc
    nhr = plan.nhr
    nch = plan.nch
    S = plan.S
    win = plan.win
    f32 = mybir.dt.float32
    bf16 = mybir.dt.bfloat16
    fp8 = mybir.dt.float8e4
    ncores = plan.n_cores
    budget = plan.budget

    nc = bacc.Bacc("TRN2", target_bir_lowering=False, debug=False,
                   enable_asserts=False, num_devices=ncores,
                   num_swdge_queues=4)

    regions = plan.regions
    nreg = len(regions)
    rb_tiles = [0] + regions
    rb_rows = [b * 64 for b in rb_tiles]
    rr_of = [rb_rows[r + 1] - rb_rows[r] for r in range(nreg)]
    SRP = plan.SRP
    OFF = plan.OFF
    chunk_par = plan.chunk_par
    chunk_reg = plan.chunk_reg
    chunk_spi = plan.chunk_spi

    if z_splits is None:
        z_splits = [nt]
    assert z_splits[-1] == nt

    oh_d = nc.dram_tensor("oh_tab", [P, nch, P], fp8, kind="ExternalInput")
    idx_d = nc.dram_tensor("idx", [P, S // 16], mybir.dt.int16,
                           kind="ExternalInput")
    invc_d = nc.dram_tensor("invc", [P, nt], f32, kind="ExternalInput")
    xsh_d = nc.dram_tensor("xsh", [2 * nhr, f], bf16, kind="ExternalInput")
    wl1_d = nc.dram_tensor("wl1", [f, hid], bf16, kind="ExternalInput")
    wr1_d = nc.dram_tensor("wr1", [f, hid], bf16, kind="ExternalInput")
    wl2p_d = nc.dram_tensor("wl2p", [hid, 64], bf16, kind="ExternalInput")
    wr2_d = nc.dram_tensor("wr2", [hid, ncl], bf16, kind="ExternalInput")
    b1_d = nc.dram_tensor("b1", [hid, 1], f32, kind="ExternalInput")
    b2_d = nc.dram_tensor("b2", [P, ncl], f32, kind="ExternalInput")
    ident_d = nc.dram_tensor("ident", [P, P], f32, kind="ExternalInput")
    identb_d = nc.dram_tensor("identb", [P, P], bf16, kind="ExternalInput")
    out_d = nc.dram_tensor("out", [npc, ncl], bf16, kind="ExternalOutput")

    with tile.TileContext(nc) as tc:
        nc.gpsimd.load_library(library_config.mlp)
        with tc.tile_pool(name="const", bufs=1) as cp, \
             tc.tile_pool(name="store", bufs=1) as sp, \
             tc.tile_pool(name="m1", bufs=m1_bufs) as mp1, \
             tc.tile_pool(name="m2", bufs=m2_bufs) as mp2, \
             tc.tile_pool(name="xr", bufs=2) as xrp, \
             tc.tile_pool(name="fin", bufs=2) as fp, \
             tc.tile_pool(name="seg", bufs=2, space="PSUM") as psum_seg, \
             tc.tile_pool(name="paux", bufs=2, space="PSUM") as psum_aux, \
             tc.tile_pool(name="px", bufs=2, space="PSUM") as psum_x, \
             tc.tile_pool(name="phT", bufs=2, space="PSUM") as psum_h, \
             tc.tile_pool(name="dram", bufs=1, space="DRAM") as dp:

            # ---- constant staging ----
            def load_const(dram, shape, dtype=f32, tag="", slices=1):
                t = cp.tile(shape, dtype, tag=tag)
                if slices == 1:
                    nc.sync.dma_start(t[:], dram[:])
                else:
                    step = shape[1] // slices
                    for i in range(slices):
                        sl = slice(i * step, (i + 1) * step if i < slices - 1 else shape[1])
                        nc.sync.dma_start(t[:, sl], dram[:, sl])
                return t

            oh_t = load_const(oh_d, [P, nch, P], fp8, tag="oh", slices=8)
            idx_t = load_const(idx_d, [P, S // 16], mybir.dt.int16, tag="idx")
            invc_t = load_const(invc_d, [P, nt], tag="invc")
            wl1_t = load_const(wl1_d, [f, hid], bf16, tag="wl1")
            wr1_t = load_const(wr1_d, [f, hid], bf16, tag="wr1")
            wl2p_t = load_const(wl2p_d, [hid, 64], bf16, tag="wl2p")
            wr2_t = load_const(wr2_d, [hid, ncl], bf16, tag="wr2")
            b1_t = load_const(b1_d, [hid, 1], tag="b1")
            b2_t = load_const(b2_d, [P, ncl], tag="b2")
            ident_t = load_const(ident_d, [P, P], tag="ident")
            identb_t = load_const(identb_d, [P, P], bf16, tag="identb")

            hT_store = sp.tile([P, nt * P], bf16, tag="hT")     # [hid, node']
            z_acc = sp.tile([P, nt, 64], bf16, tag="z_acc")     # [node', tile, zcol]
            out_acc = sp.tile([P, nt, ncl], bf16, tag="out_acc")
            s2_acc = (sp.tile([P, nt, ncl], f32, tag="s2_acc", name="s2_acc")
                      if nreg > 1 else None)

            # global chunk ids per (t, g)
            flatb = budget.reshape(-1)
            chunk_off = np.concatenate([[0], np.cumsum(flatb)])[:-1].reshape(nt, 2 * nreg)

            # collectives cannot read IO tensors: mirror xsh into internal DRAM
            xsh_i = dp.tile([2 * nhr, f], bf16, name="xsh_i")
            nc.sync.dma_start(xsh_i[:, :], xsh_d[:, :])

            for _rep in range(repeats):
                xe_full = [dp.tile([ncores, rr_of[r], P], bf16,
                                   addr_space="Shared", name=f"xe{r}")
                           for r in range(nreg)]
                xo_full = [dp.tile([ncores, rr_of[r], P], bf16,
                                   addr_space="Shared", name=f"xo{r}")
                           for r in range(nreg)]
                z2_own = dp.tile([nhr, P], bf16)
                z2_full = [dp.tile([ncores, rr_of[r], P], bf16,
                                   addr_space="Shared", name=f"z2full{r}")
                           for r in range(nreg)]

                # replicate x across cores (region-chunked)
                for r in range(nreg):
                    r0, r1 = rb_rows[r], rb_rows[r + 1]
                    nc.gpsimd.collective_compute(
                        "AllGather", mybir.AluOpType.bypass,
                        replica_groups=[list(range(ncores))],
                        ins=[xsh_i[r0:r1, :]], outs=[xe_full[r][:, :, :]])
                    nc.gpsimd.collective_compute(
                        "AllGather", mybir.AluOpType.bypass,
                        replica_groups=[list(range(ncores))],
                        ins=[xsh_i[nhr + r0:nhr + r1, :]],
                        outs=[xo_full[r][:, :, :]])

                if not do_l1:
                    continue

                x_tabs = [[xe_full[r][:, :, :].flatten_outer_dims(),
                           xo_full[r][:, :, :].flatten_outer_dims()]
                          for r in range(nreg)]
                z_tabs = [z2_full[r][:, :, :].flatten_outer_dims()
                          for r in range(nreg)]

                qrot = [0]
                msg_bufs1 = {}
                msg_bufs2 = {}

                def ensure_win(layer, r, p, w):
                    cache = msg_bufs1 if layer == 1 else msg_bufs2
                    if (r, p, w) in cache:
                        return cache[(r, p, w)]
                    lo = w * win
                    cnt = min(win, SRP[r][p] - lo)
                    pool = mp1 if layer == 1 else mp2
                    tbl = x_tabs[r][p] if layer == 1 else z_tabs[r]
                    mt = pool.tile([P, win // P, P], bf16, tag=f"m{layer}")
                    c0 = (OFF[r][p] + lo) // 16
                    nc.gpsimd.dma_gather(
                        mt[:, :cnt // P, :], tbl,
                        idx_t[:, c0:c0 + cnt // 16], cnt, cnt, P,
                        queue_num=qrot[0] % 4, single_packet=single_packet)
                    qrot[0] += 1
                    cache[(r, p, w)] = mt
                    return mt

                # ================= LAYER 1 =================
                ci = 0
                t_done = 0
                for t in range(nt):
                    ncht = int(budget[t, :].sum())
                    pt = psum_seg.tile([P, f], f32, tag="seg")
                    for j in range(ncht):
                        r = int(chunk_reg[ci]); p = int(chunk_par[ci])
                        w, col = divmod(int(chunk_spi[ci]) * P, win)
                        mt = ensure_win(1, r, p, w)
                        nc.tensor.matmul(out=pt[:], lhsT=oh_t[:, ci, :],
                                         rhs=mt[:, col // P, :],
                                         start=(j == 0), stop=(j == ncht - 1))
                        ci += 1
                    # mean scale on Act (per-partition invc), f32 out
                    aggm = fp.tile([P, f], f32, tag="aggm")
                    if ncht == 0:
                        nc.vector.memset(aggm[:], 0.0)
                    else:
                        nc.scalar.activation(out=aggm[:], in_=pt[:],
                                             func=mybir.ActivationFunctionType.Copy,
                                             scale=invc_t[:, t:t + 1])
                    paggT = psum_aux.tile([P, P], f32, tag="aux")
                    nc.tensor.transpose(out=paggT[:], in_=aggm[:], identity=ident_t[:])
                    aggT = fp.tile([P, P], bf16, tag="aggT_sb")
                    nc.scalar.activation(out=aggT[:], in_=paggT[:],
                                         func=mybir.ActivationFunctionType.Copy)
                    # own x tile -> transpose (pair layout: even rows then odd)
                    xr = xrp.tile([P, f], bf16, tag="xr")
                    nc.sync.dma_start(xr[0:64, :], xsh_d[t * 64:(t + 1) * 64, :])
                    nc.sync.dma_start(xr[64:128, :],
                                      xsh_d[nhr + t * 64:nhr + (t + 1) * 64, :])
                    pxT = psum_x.tile([P, P], bf16, tag="pxT")
                    nc.tensor.transpose(out=pxT[:], in_=xr[:], identity=identb_t[:])
                    xTt = fp.tile([P, P], bf16, tag="xT_sb")
                    nc.scalar.activation(out=xTt[:], in_=pxT[:],
                                         func=mybir.ActivationFunctionType.Copy)
                    phT = psum_h.tile([P, P], f32, tag="hT")
                    nc.tensor.matmul(out=phT[:], lhsT=wl1_t[:], rhs=aggT[:],
                                     start=True, stop=False)
                    nc.tensor.matmul(out=phT[:], lhsT=wr1_t[:], rhs=xTt[:],
                                     start=False, stop=True)
                    hT_sl = hT_store[:, t * P:(t + 1) * P]
                    nc.scalar.activation(out=hT_sl, in_=phT[:],
                                         func=mybir.ActivationFunctionType.Relu,
                                         bias=b1_t[:], scale=1.0)
                    pz = psum_aux.tile([P, P], f32, tag="aux")
                    nc.tensor.matmul(out=pz[:, 0:64], lhsT=hT_sl, rhs=wl2p_t[:],
                                     start=True, stop=True)
                    nc.scalar.activation(out=z_acc[:, t, :], in_=pz[:, 0:64],
                                         func=mybir.ActivationFunctionType.Copy)

                    t_done += 1
                    if t_done in regions:
                        k = regions.index(t_done)
                        r0, r1 = rb_rows[k], rb_rows[k + 1]
                        t0 = rb_tiles[k]
                        # z_acc[[par half], t0:t, :] -> z2_own rows, col half
                        for par, pbase in ((0, 0), (1, 64)):
                            src_ap = z_acc[pbase:pbase + 64, t0:t_done, :]
                            dst_ap = z2_own[r0:r1, pbase:pbase + 64]
                            d3 = dst_ap.rearrange("(t j) c -> j t c", j=64)
                            nc.sync.dma_start(d3, src_ap)
                        if do_ag:
                            nc.gpsimd.collective_compute(
                                "AllGather", mybir.AluOpType.bypass,
                                replica_groups=[list(range(ncores))],
                                ins=[z2_own[r0:r1, :]],
                                outs=[z2_full[k][:, :, :]])

                # ================= LAYER 2 =================
                if not do_l2:
                    continue

                def l2_chunks(t, ri):
                    ids = []
                    for g in (2 * ri, 2 * ri + 1):
                        o = int(chunk_off[t, g])
                        ids += list(range(o, o + int(budget[t, g])))
                    return ids

                # passes 0..nreg-2 accumulate region partial sums in SBUF so the
                # next region's AllGather latency hides under this pass's work
                for ri in range(nreg - 1):
                    for t in range(nt):
                        ids = l2_chunks(t, ri)
                        if not ids:
                            if ri == 0:
                                nc.vector.memset(s2_acc[:, t, :], 0.0)
                            continue
                        pt = psum_seg.tile([P, f], f32, tag="seg")
                        for j, ci2 in enumerate(ids):
                            p = int(chunk_par[ci2])
                            w, col = divmod(int(chunk_spi[ci2]) * P, win)
                            mt = ensure_win(2, ri, p, w)
                            pbase = 64 * p
                            nc.tensor.matmul(out=pt[:, 0:ncl], lhsT=oh_t[:, ci2, :],
                                             rhs=mt[:, col // P, pbase:pbase + ncl],
                                             start=(j == 0), stop=(j == len(ids) - 1))
                        if ri == 0:
                            nc.scalar.activation(out=s2_acc[:, t, :], in_=pt[:, 0:ncl],
                                                 func=mybir.ActivationFunctionType.Copy,
                                                 scale=invc_t[:, t:t + 1])
                        else:
                            s2p = fp.tile([P, ncl], f32, tag="s2")
                            nc.scalar.activation(out=s2p[:], in_=pt[:, 0:ncl],
                                                 func=mybir.ActivationFunctionType.Copy,
                                                 scale=invc_t[:, t:t + 1])
                            nc.vector.tensor_add(out=s2_acc[:, t, :],
                                                 in0=s2_acc[:, t, :], in1=s2p[:])

                rif = nreg - 1
                for t in range(nt):
                    ids = l2_chunks(t, rif)
                    pt = psum_seg.tile([P, f], f32, tag="seg")
                    for j, ci2 in enumerate(ids):
                        p = int(chunk_par[ci2])
                        w, col = divmod(int(chunk_spi[ci2]) * P, win)
                        mt = ensure_win(2, rif, p, w)
                        pbase = 64 * p
                        nc.tensor.matmul(out=pt[:, 0:ncl], lhsT=oh_t[:, ci2, :],
                                         rhs=mt[:, col // P, pbase:pbase + ncl],
                                         start=(j == 0), stop=(j == len(ids) - 1))
                    s2 = fp.tile([P, ncl], f32, tag="s2")
                    if not ids:
                        nc.vector.memset(s2[:], 0.0)
                    else:
                        nc.scalar.activation(out=s2[:], in_=pt[:, 0:ncl],
                                             func=mybir.ActivationFunctionType.Copy,
                                             scale=invc_t[:, t:t + 1])
                    po = psum_aux.tile([P, P], f32, tag="aux")
                    nc.tensor.matmul(out=po[:, 0:ncl], lhsT=hT_store[:, t * P:(t + 1) * P],
                                     rhs=wr2_t[:], start=True, stop=True)
                    otmp = fp.tile([P, ncl], f32, tag="otmp")
                    nc.vector.tensor_add(out=otmp[:], in0=po[:, 0:ncl], in1=s2[:])
                    if nreg > 1:
                        nc.vector.tensor_add(out=otmp[:], in0=otmp[:],
                                             in1=s2_acc[:, t, :])
                    # single rounding to bf16 on the final add
                    nc.vector.tensor_add(out=out_acc[:, t, :], in0=otmp[:],
                                         in1=b2_t[:])

                # output write: per-tile strided (even/odd local) rows
                for t in range(nt):
                    rows = min(P, npc - t * P)
                    n_even = (rows + 1) // 2
                    n_odd = rows // 2
                    ev = out_acc[0:n_even, t, :]
                    od = out_acc[64:64 + n_odd, t, :]
                    dst_e = out_d[t * P:t * P + 2 * n_even - 1:2, :]
                    nc.sync.dma_start(dst_e, ev)
                    if n_odd:
                        dst_o = out_d[t * P + 1:t * P + 2 * n_odd:2, :]
                        nc.sync.dma_start(dst_o, od)

    nc.compile()
    return nc


import jax
from jax.sharding import Mesh, PartitionSpec, NamedSharding
from jax.experimental.shard_map import shard_map
from concourse.bass2jax import _bass_exec_p, partition_id_tensor, install_neuronx_cc_hook


class SpmdRunner:
    def __init__(self, nc, n_cores: int):
        install_neuronx_cc_hook()
        self.nc = nc
        self.n_cores = n_cores
        partition_name = nc.partition_id_tensor.name if nc.partition_id_tensor else None
        in_names, out_names, out_avals = [], [], []
        zero_outs = []
        for alloc in nc.m.functions[0].allocations:
            if not isinstance(alloc, mybir.MemoryLocationSet):
                continue
            name = alloc.memorylocations[0].name
            if alloc.kind == "ExternalInput":
                if name != partition_name:
                    in_names.append(name)
            elif alloc.kind == "ExternalOutput":
                shape = tuple(alloc.tensor_shape)
                dtype = mybir.dt.np(alloc.dtype)
                out_names.append(name)
                out_avals.append(jax.core.ShapedArray(shape, dtype))
                zero_outs.append(np.zeros(shape, dtype))
        self.in_names = list(in_names)
        self.out_names = out_names
        self.out_avals = out_avals
        self.zero_outs = zero_outs
        n_params = len(in_names)
        all_in_names = list(in_names) + list(out_names)
        if partition_name is not None:
            all_in_names.append(partition_name)

        def _body(*args):
            operands = list(args)
            if partition_name is not None:
                operands.append(partition_id_tensor())
            outs = _bass_exec_p.bind(
                *operands,
                out_avals=tuple(out_avals),
                in_names=tuple(all_in_names),
                out_names=tuple(out_names),
                lowering_input_output_aliases=(),
                sim_require_finite=False,
                sim_require_nnan=False,
                nc=nc,
            )
            return tuple(outs)

        devices = jax.devices()[:n_cores]
        assert len(devices) == n_cores
        self.mesh = Mesh(np.asarray(devices), ("core",))
        in_specs = (PartitionSpec("core"),) * (n_params + len(out_names))
        out_specs = (PartitionSpec("core"),) * len(out_names)
        self.fn = jax.jit(
            shard_map(_body, mesh=self.mesh, in_specs=in_specs,
                      out_specs=out_specs, check_rep=False),
            keep_unused=True,
        )
        self.sharding = NamedSharding(self.mesh, PartitionSpec("core"))
        self._dev = {}
        self._zero_dev = None

    def put(self, in_maps, names):
        """device_put the given input names (concatenated across cores)."""
        n = self.n_cores
        for name in names:
            a = np.concatenate([np.asarray(in_maps[c][name]) for c in range(n)],
                               axis=0)
            self._dev[name] = jax.device_put(a, self.sharding)
        if self._zero_dev is None:
            self._zero_dev = [
                jax.device_put(
                    np.zeros((n * z.shape[0], *z.shape[1:]), z.dtype),
                    self.sharding)
                for z in self.zero_outs
            ]
        return self

    def stage(self, in_maps):
        return self.put(in_maps, self.in_names)

    def _args(self):
        return [self._dev[n] for n in self.in_names] + self._zero_dev

    def run(self):
        return self.fn(*self._args())

    def run_blocking(self):
        out = self.fn(*self._args())
        jax.block_until_ready(out)
        return out

    def results(self, out_arrs):
        n = self.n_cores
        return [
            {name: np.asarray(out_arrs[i]).reshape(n, *self.out_avals[i].shape)[c]
             for i, name in enumerate(self.out_names)}
            for c in range(n)
        ]


# ---------------- self-contained entry point ----------------
_CACHE = {}

def kernel(**inputs):
    import numpy as _np
    x = _np.asarray(inputs["x"], dtype=_np.float32)
    edge_index = _np.asarray(inputs["edge_index"])
    Wl1 = _np.asarray(inputs["Wl1"], dtype=_np.float32)
    Wr1 = _np.asarray(inputs["Wr1"], dtype=_np.float32)
    b1 = _np.asarray(inputs["b1"], dtype=_np.float32)
    Wl2 = _np.asarray(inputs["Wl2"], dtype=_np.float32)
    Wr2 = _np.asarray(inputs["Wr2"], dtype=_np.float32)
    b2 = _np.asarray(inputs["b2"], dtype=_np.float32)
    N, F = x.shape
    H = Wl1.shape[1]
    C = Wl2.shape[1]
    import hashlib
    eh = hashlib.md5(edge_index.tobytes()).hexdigest()
    key = ("plan", N, F, H, C, edge_index.shape[1], eh)
    if key not in _CACHE:
        plan = make_plan(edge_index, N, F, H, C, 8)
        nc = build_program(plan)
        runner = SpmdRunner(nc, 8)
        runner.put(stage_const(plan), CONST_NAMES)
        _CACHE[key] = (plan, runner)
    plan, runner = _CACHE[key]
    runner.put(stage_x(plan, x, Wl1, Wr1, b1, Wl2, Wr2, b2), X_NAMES)
    out_arrs = runner.run_blocking()
    results = runner.results(out_arrs)
    out = _np.concatenate([results[c]["out"] for c in range(8)], axis=0)
    return out[:N].astype(_np.float32)
